# revision 32
# baseline (speedup 1.0000x reference)
"""Distributed Trainium2 Bass kernel for a dense-transformer attention block.

Sharding (8 NeuronCores): core cid = 4*b + g
  - b = batch index (B=2), g = kv-head group (N_KV_HEADS=4)
  - Each core: LN1(x[b]) -> its 4 query heads + its 1 kv head (column
    parallel wq/wk/wv), RoPE, causal GQA attention with pre-ictal bias,
    per-q-tile AllGather of attention outputs (groups [0..3], [4..7]),
    LN2 via gathered partial stats, column-parallel wo -> output columns
    [512g:512g+512].  Host concatenates the 8 output shards.

Schedule: everything is emitted interleaved so the PE never idles
(TRN2 PE p-state drops to half clock after any bubble):
  - mask build first (needs only the 8KB labels tensor), overlapping the
    x input DMA;
  - LN1 per 4-tile group as x lands (stats split DVE bn_stats / ACT
    accum-sums), then that group's K/V/Q projections + RoPE;
  - attention per q-tile t right after its kv group is projected
    (2 heads per pass so PSUM fits), diag/prev-diag bias+causal mask
    preloaded into PSUM via ident matmuls;
  - per-tile outputs are transposed pre-gather (4x cheaper than post)
    and AllGathered together with the tile's partial LN2 bn_stats;
  - wo runs on the RAW gathered attention:  out = rs*(a@W) - rs*mu*s + ob
    (s = colsums of W), so the output projection never waits for the
    LN2 normalization; the per-row rs/mu correction is applied after.
  - rsqrt = DVE reciprocal + ACT Sqrt batched per 4 tiles (avoids
    activation-table thrash against the softmax Exp).

Compute dtype: bf16 matmul operands, f32 PSUM accumulation, f32 softmax/LN.
"""

import math
from contextlib import ExitStack

import numpy as np
import ml_dtypes

import concourse.bass as bass
import concourse.bacc as bacc
import concourse.mybir as mybir
import concourse.tile as tile
from concourse.bass_utils import run_bass_kernel_spmd

# Problem constants (hardcoded per spec nn_Attention_36120674959366)
B = 2
S = 2048
DIM = 2048
N_HEADS = 16
N_KV_HEADS = 4
HEAD_DIM = 128
NH_LOC = N_HEADS // N_KV_HEADS  # 4 q-heads per core
DQ_LOC = NH_LOC * HEAD_DIM      # 512
PRE_ICTAL_WINDOW = 10
PRE_ICTAL_BIAS = 2.0
LN_EPS = 1e-5
NEG_INF = -1e9

SQD = math.sqrt(HEAD_DIM)           # 11.3137085
INV_SQD = 1.0 / SQD
BIAS_SCALED = PRE_ICTAL_BIAS * SQD  # 22.627417
NEG_SCALED = NEG_INF * SQD          # -1.13137085e10

NT = S // 128                        # 16 tiles of 128 rows
NC = DIM // 128                      # 16 dim chunks

F32 = mybir.dt.float32
BF16 = mybir.dt.bfloat16

# bounce buffer layout: [attn_cT: 4*128*128 bf16][st6: 128*6 f32 = 1536 bf16]
BNC_DATA = 4 * 128 * 128             # 65536 bf16 elements
BNC_ST6 = 128 * 6 * 2                # f32 viewed as 2x bf16
BNC_TOT = BNC_DATA + BNC_ST6

WO_LAG = 7

_CACHED = {}


def build_nc():
    nc = bacc.Bacc("TRN2", target_bir_lowering=False, debug=False, num_devices=8)

    # ---- kernel I/O (per-core shards; same graph on all 8 cores) ----
    xs = nc.dram_tensor("xs", [S, DIM], BF16, kind="ExternalInput")
    wqT = nc.dram_tensor("wqT", [128, NC * DQ_LOC], BF16, kind="ExternalInput")
    wkT = nc.dram_tensor("wkT", [128, NC * HEAD_DIM], BF16, kind="ExternalInput")
    wvT = nc.dram_tensor("wvT", [128, NC * HEAD_DIM], BF16, kind="ExternalInput")
    woT = nc.dram_tensor("woT", [128, NC * DQ_LOC], BF16, kind="ExternalInput")
    qb = nc.dram_tensor("qb", [128, NH_LOC], F32, kind="ExternalInput")
    kb = nc.dram_tensor("kb", [128, 1], F32, kind="ExternalInput")
    vbt = nc.dram_tensor("vbt", [128, HEAD_DIM], BF16, kind="ExternalInput")
    obt = nc.dram_tensor("obt", [128, DQ_LOC], F32, kind="ExternalInput")
    sbt = nc.dram_tensor("sbt", [128, DQ_LOC], F32, kind="ExternalInput")
    labels = nc.dram_tensor("labels", [S], F32, kind="ExternalInput")
    cosT = nc.dram_tensor("cosT", [HEAD_DIM, S], BF16, kind="ExternalInput")
    sinT = nc.dram_tensor("sinT", [HEAD_DIM, S], BF16, kind="ExternalInput")
    ident = nc.dram_tensor("ident", [128, 128], BF16, kind="ExternalInput")
    ident32 = nc.dram_tensor("ident32", [128, 128], F32, kind="ExternalInput")
    pswap = nc.dram_tensor("pswap", [128, 128], BF16, kind="ExternalInput")
    out = nc.dram_tensor("out", [S, DQ_LOC], BF16, kind="ExternalOutput")

    AF = mybir.ActivationFunctionType
    OP = mybir.AluOpType

    with tile.TileContext(nc) as tc, ExitStack() as st:
        pc = st.enter_context(tc.tile_pool(name="const", bufs=1))
        dr = st.enter_context(tc.tile_pool(name="dr", bufs=1, space="DRAM"))
        ps = st.enter_context(tc.tile_pool(name="ps", bufs=1, space="PSUM"))

        def meg(name):
            return ps.tile([128, 512], F32, tag="meg", bufs=3, name=name)

        def big(name):
            return meg(name)[:, 0:512]

        def pv2(name):
            # two packed [129]-wide softmax-PV accumulators in one bank
            return ps.tile([128, 320], F32, tag="pv2", bufs=3, name=name)

        def tpt(name):
            return ps.tile([128, 512], BF16, tag="tp", bufs=2, name=name)

        # ================= input DMAs =================
        # tiny consts + labels first (mask build starts immediately), then
        # x tiles interleaved with weights so LN1 streams while weights land.
        lab_sb = pc.tile([1, S], F32, tag="lab")
        nc.sync.dma_start(out=lab_sb[:, :],
                          in_=labels.ap().rearrange("(o s) -> o s", o=1))
        ident_sb = pc.tile([128, 128], BF16, tag="ident")
        nc.sync.dma_start(out=ident_sb[:, :], in_=ident[:, :])
        ident32_sb = pc.tile([128, 128], F32, tag="ident32")
        nc.sync.dma_start(out=ident32_sb[:, :], in_=ident32[:, :])
        pswap_sb = pc.tile([128, 128], BF16, tag="pswap")
        nc.sync.dma_start(out=pswap_sb[:, :], in_=pswap[:, :])
        qb_sb = pc.tile([128, NH_LOC], F32, tag="qb")
        nc.sync.dma_start(out=qb_sb[:, :], in_=qb[:, :])
        kb_sb = pc.tile([128, 1], F32, tag="kb")
        nc.sync.dma_start(out=kb_sb[:, :], in_=kb[:, :])
        vb_sb = pc.tile([128, HEAD_DIM], BF16, tag="vb")
        nc.sync.dma_start(out=vb_sb[:, :], in_=vbt[:, :])

        # x tiles stream on the Scalar engine's DMA queue so the weight
        # loads (sync queue) and mask ops (gpsimd/DVE) never stall them.
        pxt = st.enter_context(tc.tile_pool(name="pxt", bufs=1))
        xt_tiles = []
        for i in range(NT):
            xt = pxt.tile([128, DIM], BF16, tag="xt", bufs=6, name=f"xt{i}")
            xt_tiles.append(xt)

        def xt_dma(i):
            nc.scalar.dma_start(out=xt_tiles[i][:, :],
                                in_=xs[128 * i:128 * i + 128, :])

        for i in range(6):
            xt_dma(i)
        cos_sb = pc.tile([128, S], BF16, tag="cos")
        nc.sync.dma_start(out=cos_sb[:, :], in_=cosT[:, :])
        sin_sb = pc.tile([128, S], BF16, tag="sin")
        nc.sync.dma_start(out=sin_sb[:, :], in_=sinT[:, :])

        pw = st.enter_context(tc.tile_pool(name="qkvw", bufs=1))
        wk_sb = pw.tile([128, NC, HEAD_DIM], BF16, tag="wk")
        nc.sync.dma_start(
            out=wk_sb.rearrange("p c n -> p (c n)"), in_=wkT[:, :])
        wv_sb = pw.tile([128, NC, HEAD_DIM], BF16, tag="wv")
        nc.sync.dma_start(
            out=wv_sb.rearrange("p c n -> p (c n)"), in_=wvT[:, :])
        wq_sb = pw.tile([128, NC, DQ_LOC], BF16, tag="wq")
        nc.sync.dma_start(
            out=wq_sb.rearrange("p c n -> p (c n)"), in_=wqT[:, :])
        ob_sb = pc.tile([128, DQ_LOC], F32, tag="ob")
        nc.sync.dma_start(out=ob_sb[:, :], in_=obt[:, :])
        s_sb = pc.tile([128, DQ_LOC], F32, tag="s")
        nc.sync.dma_start(out=s_sb[:, :], in_=sbt[:, :])
        wo_sb = pc.tile([128, NC, DQ_LOC], BF16, tag="wo")
        nc.sync.dma_start(
            out=wo_sb.rearrange("p c n -> p (c n)"), in_=woT[:, :])

        ones_col = pc.tile([1, 128], BF16, tag="ones_col")
        nc.vector.memset(ones_col[:, :], 1.0)

        # ============ mask build ====
        # mgd[kt]: diag-tile bias+causal for q-tile kt; mgp[kt]: bias for
        # q-tile kt+1 vs key-tile kt.  [k partitions, q free], pre-scaled
        # by sqrt(d) (softmax exp applies 1/sqrt(d) to the whole PSUM).
        # The label scan is emitted after group-0 LN1 (keeps the DVE free
        # at t=0); per-tile mask slots are built just-in-time, one tile
        # ahead of the attention that consumes them.
        mgd = pc.tile([128, NT, 128], BF16, tag="mgd")
        mgp = pc.tile([128, NT, 128], BF16, tag="mgp")
        colv = pc.tile([128, NT], F32, tag="colv")
        # cumsum counts are small ints -> exact in bf16
        csrow = pc.tile([1, S + 12], BF16, tag="csrow")

        def emit_scan():
            zrow = pcs.tile([1, S], BF16, tag="zrow")
            nc.vector.memset(zrow[:, :], 0.0)
            nc.vector.memset(csrow[:, 0:1], 0.0)
            nc.vector.tensor_tensor_scan(
                out=csrow[:, 1:S + 1], data0=lab_sb[:, :], data1=zrow[:, :],
                initial=0.0, op0=OP.add, op1=OP.add)
            for j in range(11):
                nc.vector.tensor_copy(csrow[:, S + 1 + j:S + 2 + j],
                                      csrow[:, S:S + 1])
            # spill to DRAM, reload shifted as [NT,128], transpose via PE:
            #   colv[p, kt] = cs[min(k+10, S-1)], k = 128*kt + p
            csbuf = dr.tile([S + 12], BF16)
            nc.sync.dma_start(out=csbuf.rearrange("(o s) -> o s", o=1),
                              in_=csrow[:, :])
            cs16 = pcs.tile([NT, 128], BF16, tag="cs16")
            nc.sync.dma_start(
                out=cs16[:, :],
                in_=csbuf[11:11 + S].rearrange("(t p) -> t p", p=128))
            colv_ps = big("colv_ps")
            nc.tensor.matmul(colv_ps[:, 0:NT], lhsT=cs16[:, :],
                             rhs=ident_sb[0:NT, 0:NT], start=True, stop=True)
            nc.vector.tensor_copy(colv[:, :], colv_ps[:, 0:NT])
            pcs.release()

        def emit_mask(t):
            # rb[p, j] = cs[128*t + j - 1]  (q coordinates of tile t)
            rb = big(f"rb{t}")
            nc.tensor.matmul(rb[:, 0:128], lhsT=ones_col[:, :],
                             rhs=csrow[:, 128 * t:128 * t + 128],
                             start=True, stop=True)
            sl = mgd[:, t, :]
            nc.vector.tensor_scalar(
                out=sl, in0=rb[:, 0:128],
                scalar1=colv[:, t:t + 1], scalar2=BIAS_SCALED,
                op0=OP.is_lt, op1=OP.mult)
            nc.gpsimd.affine_select(
                out=sl, in_=sl,
                compare_op=OP.is_ge, fill=NEG_SCALED,
                base=0, channel_multiplier=-1, pattern=[[1, 128]])
            if t >= 1:
                sl = mgp[:, t - 1, :]
                nc.vector.tensor_scalar(
                    out=sl, in0=rb[:, 0:128],
                    scalar1=colv[:, t - 1:t], scalar2=BIAS_SCALED,
                    op0=OP.is_lt, op1=OP.mult)

        # ================= persistent attention tensors =================
        pqkv = st.enter_context(tc.tile_pool(name="qkv", bufs=1))
        qT = pqkv.tile([128, NH_LOC, S], BF16, tag="qT")
        kT = pqkv.tile([128, S], BF16, tag="kT")
        v_aug = pqkv.tile([128, NT, 132], BF16, tag="v_aug")
        nc.gpsimd.memset(v_aug[:, :, 128:129], 1.0)

        p1 = st.enter_context(tc.tile_pool(name="ln1t", bufs=1))
        ptmp = st.enter_context(tc.tile_pool(name="ln1tmp", bufs=1))
        prope = st.enter_context(tc.tile_pool(name="rope", bufs=1))
        pat = st.enter_context(tc.tile_pool(name="attn", bufs=1))
        pln2 = st.enter_context(tc.tile_pool(name="ln2", bufs=1))
        pcs = tc.alloc_tile_pool(name="csum", bufs=1)

        # ================= LN1 for one row-tile =================
        def emit_ln1(i):
            if i + 6 < NT:
                xt_dma(i + 6)
            xt = xt_tiles[i]
            mu = ptmp.tile([128, 1], F32, tag="mu", bufs=4, name=f"mu{i}")
            var = ptmp.tile([128, 1], F32, tag="var", bufs=4)
            if i % 2 == 0:
                st6 = ptmp.tile([128, 4, 6], F32, tag="st6", bufs=2)
                for a4 in range(4):
                    nc.vector.bn_stats(st6[:, a4, :],
                                       xt[:, 512 * a4:512 * a4 + 512])
                mv = ptmp.tile([128, 2], F32, tag="mv", bufs=2)
                nc.vector.bn_aggr(mv[:, :], st6[:, :, :])
                nc.vector.tensor_copy(mu[:, :], mv[:, 0:1])
                nc.vector.tensor_copy(var[:, :], mv[:, 1:2])
            else:
                scr = ptmp.tile([128, DIM], BF16, tag="scr", bufs=1)
                s1 = ptmp.tile([128, 1], F32, tag="s1", bufs=2)
                s2 = ptmp.tile([128, 1], F32, tag="s2", bufs=2)
                nc.scalar.activation(scr[:, :], xt[:, :], AF.Copy,
                                     accum_out=s1[:, :])
                nc.scalar.activation(scr[:, :], xt[:, :], AF.Square,
                                     accum_out=s2[:, :])
                nc.vector.tensor_scalar_mul(mu[:, :], s1[:, :], 1.0 / DIM)
                musq = ptmp.tile([128, 1], F32, tag="musq", bufs=2)
                nc.vector.tensor_tensor(musq[:, :], mu[:, :], mu[:, :],
                                        op=OP.mult)
                nc.vector.scalar_tensor_tensor(
                    out=var[:, :], in0=s2[:, :], scalar=1.0 / DIM,
                    in1=musq[:, :], op0=OP.mult, op1=OP.subtract)
            rv = ptmp.tile([128, 1], F32, tag="rv", bufs=4, name=f"rv{i}")
            nc.vector.tensor_scalar_add(rv[:, :], var[:, :], LN_EPS)
            nc.vector.reciprocal(rv[:, :], rv[:, :])
            return mu, rv

        # ================= projections + RoPE for one sg strip ==========
        def rope_block(dst, w_sb, h, sg, lnT):
            # dst: [128, 512] slice of qT/kT; raw = W.T@ln1T + bias, then
            # rotary combine.  pswap output consumed from PSUM by DVE.
            lo = 512 * sg
            pq = big(f"pq_{sg}_{h}")
            for c in range(NC):
                lhsT = w_sb[:, c, :] if h is None \
                    else w_sb[:, c, 128 * h:128 * h + 128]
                nc.tensor.matmul(pq, lhsT=lhsT,
                                 rhs=lnT[:, c, :],
                                 start=(c == 0), stop=(c == NC - 1))
            raw = prope.tile([128, 512], BF16, tag="raw", bufs=2)
            bias_ap = kb_sb[:, 0:1] if h is None else qb_sb[:, h:h + 1]
            nc.scalar.activation(raw[:, :], pq, AF.Identity,
                                 bias=bias_ap)
            pw2 = big(f"pw2_{sg}_{h}")
            nc.tensor.matmul(pw2, lhsT=pswap_sb[:, :], rhs=raw[:, :],
                             start=True, stop=True)
            t1 = prope.tile([128, 512], BF16, tag="t1", bufs=2)
            nc.vector.tensor_mul(t1[:, :], raw[:, :], cos_sb[:, lo:lo + 512])
            t2 = prope.tile([128, 512], BF16, tag="t2", bufs=2)
            nc.vector.tensor_tensor(t2[:, :], pw2,
                                    sin_sb[:, lo:lo + 512], op=OP.mult)
            nc.vector.tensor_add(dst, t1[:, :], t2[:, :])

        def emit_half(g4, half, lnT):
            # LN1 + transposes for tiles 4*g4+2*half .. +1 (DVE tile then
            # ACT tile run on separate engines in parallel)
            murs = [emit_ln1(4 * g4 + 2 * half + j) for j in range(2)]
            rss = []
            for j in range(2):
                rs = ptmp.tile([128, 1], F32, tag="rs", bufs=4,
                               name=f"rs{4 * g4 + 2 * half + j}")
                nc.scalar.activation(rs[:, :], murs[j][1][:, :], AF.Sqrt)
                rss.append(rs)
            xh_tiles = []
            for j in range(2):
                i = 4 * g4 + 2 * half + j
                xh = ptmp.tile([128, DIM], BF16, tag="xh", bufs=4)
                nc.vector.tensor_scalar(
                    out=xh[:, :], in0=xt_tiles[i][:, :],
                    scalar1=murs[j][0][:, :], scalar2=rss[j][:, :],
                    op0=OP.subtract, op1=OP.mult)
                xh_tiles.append(xh)
            for c in range(NC):
                pt = tpt(f"lt_{g4}_{half}_{c}")
                for j in range(2):
                    nc.tensor.transpose(
                        pt[:, 128 * j:128 * j + 128],
                        xh_tiles[j][:, 128 * c:128 * c + 128],
                        ident_sb[:, :])
                nc.scalar.activation(lnT[:, c, 256 * half:256 * half + 256],
                                     pt[:, 0:256], AF.Copy)

        def emit_group_proj(g4):
            lnT = p1.tile([128, NC, 512], BF16, tag="ln1T", bufs=1,
                          name=f"ln1T{g4}")
            emit_half(g4, 0, lnT)
            emit_half(g4, 1, lnT)
            # K strip
            rope_block(kT[:, 512 * g4:512 * g4 + 512], wk_sb, None, g4, lnT)
            # V tiles
            for j4 in range(4):
                i = 4 * g4 + j4
                pv = big(f"pv_{i}")
                for c in range(NC):
                    nc.tensor.matmul(
                        pv[:, 0:128],
                        lhsT=lnT[:, c, 128 * j4:128 * j4 + 128],
                        rhs=wv_sb[:, c, :],
                        start=(c == 0), stop=(c == NC - 1))
                nc.vector.tensor_add(v_aug[:, i, 0:128], pv[:, 0:128],
                                     vb_sb[:, :])
            # Q strips
            for h in range(NH_LOC):
                rope_block(qT[:, h, 512 * g4:512 * g4 + 512], wq_sb, h,
                           g4, lnT)

        # ================= attention for one q-tile =================
        bounce_ins = [None] * (NT // 2)
        bounce_outs = [None] * (NT // 2)
        bounce_ins_1 = {}
        bounce_outs_1 = {}
        attn_cs = {}

        def emit_attn(t):
            attn_c = pat.tile([128, 4, 128], BF16, tag="attn_c", bufs=3,
                              name=f"attn_c{t}")
            pva = pv2(f"pva_{t}")
            pvb = pv2(f"pvb_{t}")
            # four packed softmax-PV accumulators: (bank, region) per head
            pvp = [pva[:, 0:129], pva[:, 160:289],
                   pvb[:, 0:129], pvb[:, 160:289]]
            def emit_qk(k):
                mega = meg(f"mega_{t}_{k}")
                if k >= t - 1:
                    mt = mgd if k == t else mgp
                    for hp in range(4):
                        nc.tensor.matmul(
                            mega[:, 128 * hp:128 * hp + 128],
                            lhsT=ident_sb[:, :],
                            rhs=mt[:, k, :],
                            start=(hp == 0), stop=False,
                            skip_group_check=True)
                    nc.tensor.matmul(
                        mega, lhsT=kT[:, 128 * k:128 * k + 128],
                        rhs=qT[:, :, 128 * t:128 * t + 128],
                        start=False, stop=True,
                        skip_group_check=True)
                else:
                    nc.tensor.matmul(
                        mega, lhsT=kT[:, 128 * k:128 * k + 128],
                        rhs=qT[:, :, 128 * t:128 * t + 128],
                        start=True, stop=True)
                return mega

            def emit_exp(k, mega):
                pt = pat.tile([128, 512], BF16, tag="pt_sm", bufs=3)
                nc.scalar.activation(pt[:, :], mega, AF.Exp, scale=INV_SQD)
                return pt

            def emit_pv(k, pt):
                for hp in range(4):
                    # one accumulation group per PSUM bank: start only
                    # on the bank's very first write (h even, k==0)
                    nc.tensor.matmul(
                        pvp[hp],
                        lhsT=pt[:, 128 * hp:128 * hp + 128],
                        rhs=v_aug[:, k, 0:129],
                        start=(k == 0 and hp % 2 == 0),
                        stop=(k == t and hp % 2 == 1),
                        skip_group_check=True)

            # software-pipelined: QK(k+1) issues on the PE before PV(k)
            # so the PE never waits for exp(k) on the ACT engine.
            prev = None
            for k in range(t + 1):
                mega = emit_qk(k)
                pt = emit_exp(k, mega)
                if prev is not None:
                    emit_pv(*prev)
                prev = (k, pt)
            emit_pv(*prev)
            for hp in range(4):
                rcp = pat.tile([128, 1], F32, tag="rcp", bufs=4)
                nc.vector.reciprocal(rcp[:, :], pvp[hp][:, 128:129])
                nc.vector.tensor_single_scalar(
                    out=attn_c[:, hp, :],
                    in_=pvp[hp][:, 0:128],
                    scalar=rcp[:, :], op=OP.mult)
            attn_cs[t] = attn_c

        def emit_attn_flush(t):
            # deferred one tile so the PE transposes never head-of-line
            # block behind the tile's DVE normalize chain
            attn_c = attn_cs.pop(t)
            # partial LN2 stats for this core's 512 columns
            st6a = pat.tile([128, 6], F32, tag="st6a", bufs=2)
            nc.vector.bn_stats(st6a[:, :],
                               attn_c.rearrange("p h v -> p (h v)"))
            st6b = pat.tile([128, 6, 2], BF16, tag="st6b", bufs=2)
            nc.vector.tensor_copy(
                st6b.rearrange("p a b -> p (a b)").bitcast(F32),
                st6a[:, :])
            # pre-gather transpose: [q, (h,v)] -> [(h,v), q] in 128-chunks
            tp = tpt(f"at_{t}")
            for c4 in range(4):
                nc.tensor.transpose(tp[:, 128 * c4:128 * c4 + 128],
                                    attn_c[:, c4, :], ident_sb[:, :])
            attn_cT = pat.tile([128, 4, 128], BF16, tag="attn_cT", bufs=2)
            nc.vector.tensor_copy(
                attn_cT.rearrange("p c q -> p (c q)"), tp[:, :])
            if t >= 14:
                bounce_ins_1[t] = dr.tile([BNC_TOT], BF16, name=f"bin1_{t}")
                bounce_in = bounce_ins_1[t]
                off = 0
            else:
                if t % 2 == 0:
                    bounce_ins[t // 2] = dr.tile([2 * BNC_TOT], BF16,
                                                 name=f"bin{t // 2}")
                bounce_in = bounce_ins[t // 2]
                off = (t % 2) * BNC_TOT
            nc.sync.dma_start(
                out=bounce_in[off:off + BNC_DATA].rearrange(
                    "(c p q) -> p c q", c=4, p=128),
                in_=attn_cT[:, :, :])
            nc.gpsimd.dma_start(
                out=bounce_in[off + BNC_DATA:off + BNC_TOT].rearrange(
                    "(p n) -> p n", p=128),
                in_=st6b.rearrange("p a b -> p (a b)"))

        def emit_ag(pr):
            bounce_out = dr.tile([4, 2 * BNC_TOT], BF16, name=f"bout{pr}")
            nc.gpsimd.collective_compute(
                "AllGather",
                mybir.AluOpType.bypass,
                replica_groups=[[0, 1, 2, 3], [4, 5, 6, 7]],
                ins=[bounce_ins[pr][:].opt()],
                outs=[bounce_out[:, :].opt()],
            )
            bounce_outs[pr] = bounce_out

        def emit_ag_single(t):
            # single-tile gather (used for the last tiles to shrink the
            # tail exposure of the final collective)
            bounce_out = dr.tile([4, BNC_TOT], BF16, name=f"bout1_{t}")
            nc.gpsimd.collective_compute(
                "AllGather",
                mybir.AluOpType.bypass,
                replica_groups=[[0, 1, 2, 3], [4, 5, 6, 7]],
                ins=[bounce_ins_1[t][:].opt()],
                outs=[bounce_out[:, :].opt()],
            )
            bounce_outs_1[t] = bounce_out

        # ============ gathered-side LN2 stats + raw wo ============
        rvs = {}
        posbs = {}
        prefetched = {}

        def emit_prefetch(t):
            if t >= 14:
                bo = bounce_outs_1[t]
                off = 0
            else:
                bo = bounce_outs[t // 2]
                off = (t % 2) * BNC_TOT
            afsT = pln2.tile([128, 16, 128], BF16, tag="afsT", bufs=3,
                             name=f"afsT{t}")
            for g in range(4):
                nc.sync.dma_start(
                    out=afsT[:, 4 * g:4 * g + 4, :],
                    in_=bo[g, off:off + BNC_DATA].rearrange(
                        "(c p q) -> p c q", c=4, p=128))
            st6g = pln2.tile([128, 4, 6, 2], BF16, tag="st6g", bufs=3)
            nc.gpsimd.dma_start(
                out=st6g.rearrange("p g a b -> p g (a b)"),
                in_=bo[:, off + BNC_DATA:off + BNC_TOT].rearrange(
                    "g (p n) -> p g n", p=128))
            prefetched[t] = (afsT, st6g)

        def emit_ln2mm(t):
            afsT, st6g = prefetched.pop(t)
            st6f = pln2.tile([128, 4, 6], F32, tag="st6f", bufs=2)
            nc.vector.tensor_copy(
                st6f.rearrange("p g a -> p (g a)"),
                st6g.rearrange("p g a b -> p (g a b)").bitcast(F32))
            mv2 = pln2.tile([128, 2], F32, tag="mv2", bufs=2)
            nc.vector.bn_aggr(mv2[:, :], st6f[:, :, :])
            rv2 = pln2.tile([128, 1], F32, tag="rv2", bufs=9, name=f"rv2_{t}")
            nc.vector.tensor_scalar_add(rv2[:, :], mv2[:, 1:2], LN_EPS)
            nc.vector.reciprocal(rv2[:, :], rv2[:, :])
            rvs[t] = rv2
            nmu = pln2.tile([128, 1], F32, tag="nmu", bufs=2)
            nc.vector.tensor_scalar_mul(nmu[:, :], mv2[:, 0:1], -1.0)
            po = big(f"po_{t}")
            for c in range(NC):
                nc.tensor.matmul(po, lhsT=afsT[:, c, :],
                                 rhs=wo_sb[:, c, :],
                                 start=(c == 0), stop=(c == NC - 1))
            posb = pln2.tile([128, DQ_LOC], BF16, tag="posb", bufs=10,
                             name=f"posb{t}")
            nc.vector.scalar_tensor_tensor(
                out=posb[:, :], in0=s_sb[:, :], scalar=nmu[:, :],
                in1=po, op0=OP.mult, op1=OP.add)
            posbs[t] = posb

        def emit_finish(ts):
            # batched ACT Sqrt (one table switch per batch), then the
            # rs correction + output DMA for each tile in the batch.
            rss = {}
            for t in ts:
                rs = pln2.tile([128, 1], F32, tag="rs2", bufs=9,
                               name=f"rs2_{t}")
                nc.scalar.activation(rs[:, :], rvs[t][:, :], AF.Sqrt)
                rss[t] = rs
            for t in ts:
                osb = pln2.tile([128, DQ_LOC], BF16, tag="osb", bufs=2,
                                name=f"osb{t}")
                nc.vector.scalar_tensor_tensor(
                    out=osb[:, :], in0=posbs[t][:, :], scalar=rss[t][:, :],
                    in1=ob_sb[:, :], op0=OP.mult, op1=OP.add)
                nc.gpsimd.dma_start(out=out[128 * t:128 * t + 128, :],
                                    in_=osb[:, :])

        # ================= main interleaved schedule =================
        # tiny dummy collective up front: absorbs the one-time CC ring
        # warmup (~20us) while the PE is still in LN1/proj.
        warm_in = dr.tile([2 * BNC_TOT], BF16, name="warm_in")
        warm_sb = pat.tile([128, 1048], BF16, tag="warm")
        nc.vector.memset(warm_sb[:, :], 0.0)
        nc.gpsimd.dma_start(
            out=warm_in.rearrange("(p n) -> p n", p=128),
            in_=warm_sb[:, :])
        warm_out = dr.tile([4, 2 * BNC_TOT], BF16, name="warm_out")
        nc.gpsimd.collective_compute(
            "AllGather", mybir.AluOpType.bypass,
            replica_groups=[[0, 1, 2, 3], [4, 5, 6, 7]],
            ins=[warm_in[:].opt()], outs=[warm_out[:, :].opt()])

        for g4 in range(4):
            emit_group_proj(g4)
            if g4 == 0:
                emit_scan()
                emit_mask(0)
                emit_mask(1)
            for j4 in range(4):
                t = 4 * g4 + j4
                if t + 2 < NT:
                    emit_mask(t + 2)
                emit_attn(t)
                if t >= 1:
                    emit_attn_flush(t - 1)
                    if t == 15:
                        emit_ag_single(14)
                    elif t % 2 == 0:
                        emit_ag((t - 1) // 2)
                if t >= WO_LAG - 2:
                    emit_prefetch(t - WO_LAG + 2)
                if t >= WO_LAG:
                    emit_ln2mm(t - WO_LAG)
                if t - WO_LAG in (3, 7):
                    emit_finish(range(t - WO_LAG - 3, t - WO_LAG + 1))
        emit_attn_flush(15)
        emit_ag_single(15)
        emit_prefetch(11)
        emit_ln2mm(9)
        emit_prefetch(12)
        emit_ln2mm(10)
        emit_prefetch(13)
        emit_ln2mm(11)
        emit_prefetch(14)
        emit_ln2mm(12)
        emit_finish(range(8, 12))
        emit_prefetch(15)
        emit_ln2mm(13)
        emit_ln2mm(14)
        emit_ln2mm(15)
        emit_finish(range(12, 16))

    nc.compile()
    return nc


def _prep_inputs(x, freqs_cis, seizure_labels, wq, wk, wv, wo,
                 ln1_w, ln1_b, ln2_w, ln2_b):
    bf16 = ml_dtypes.bfloat16
    cos = np.asarray(freqs_cis[..., 0], dtype=np.float32)  # [S, 64]
    sin = np.asarray(freqs_cis[..., 1], dtype=np.float32)
    cosT = np.ascontiguousarray(np.repeat(cos.T, 2, axis=0), dtype=bf16)
    sgn = np.where(np.arange(HEAD_DIM) % 2 == 0, -1.0, 1.0).astype(np.float32)
    sinT = np.ascontiguousarray(np.repeat(sin.T, 2, axis=0) * sgn[:, None],
                                dtype=bf16)
    ident = np.eye(128, dtype=bf16)
    ident32 = np.eye(128, dtype=np.float32)
    psw = np.zeros((128, 128), dtype=np.float32)
    idx = np.arange(128)
    psw[idx ^ 1, idx] = 1.0  # out[m, s] = sum_k psw[k, m] * in[k, s] = in[m^1, s]
    psw = psw.astype(bf16)

    # fold LN affine weights into the projection weights (host-side
    # preprocessing, standard inference-time weight folding):
    #   ln(x)@W.T = xhat@(W*w).T + b@W.T
    w1 = np.asarray(ln1_w, np.float64)
    b1 = np.asarray(ln1_b, np.float64)
    w2 = np.asarray(ln2_w, np.float64)
    b2 = np.asarray(ln2_b, np.float64)
    in_maps = []
    for cid in range(8):
        b, g = divmod(cid, 4)
        wq_s = np.asarray(wq[DQ_LOC * g:DQ_LOC * (g + 1), :], np.float64)
        wk_s = np.asarray(wk[HEAD_DIM * g:HEAD_DIM * (g + 1), :], np.float64)
        wv_s = np.asarray(wv[HEAD_DIM * g:HEAD_DIM * (g + 1), :], np.float64)
        wo_s = np.asarray(wo[DQ_LOC * g:DQ_LOC * (g + 1), :], np.float64)
        qb_v = (b1 @ wq_s.T).astype(np.float32)         # [512]
        kb_v = (b1 @ wk_s.T).astype(np.float32)         # [128]
        vb_v = (b1 @ wv_s.T).astype(np.float32)         # [128]
        ob_v = (b2 @ wo_s.T).astype(np.float32)         # [512]
        wo_eff = wo_s * w2                              # [512, 2048]
        s_v = wo_eff.sum(axis=1).astype(np.float32)     # [512] colsums of W.T
        def arr(wT):  # [DIM, n] -> [128, NC*n] chunk-major per partition
            n = wT.shape[1]
            return np.ascontiguousarray(
                wT.reshape(NC, 128, n).transpose(1, 0, 2).reshape(128, NC * n),
                dtype=bf16)
        in_maps.append({
            "xs": np.ascontiguousarray(x[b], dtype=bf16),
            "wqT": arr((wq_s * w1).T),
            "wkT": arr((wk_s * w1).T),
            "wvT": arr((wv_s * w1).T),
            "woT": arr(wo_eff.T),
            "qb": np.ascontiguousarray(
                qb_v.reshape(NH_LOC, 128).T, dtype=np.float32),
            "kb": np.ascontiguousarray(kb_v.reshape(128, 1), dtype=np.float32),
            "vbt": np.ascontiguousarray(np.tile(vb_v, (128, 1)), dtype=bf16),
            "obt": np.ascontiguousarray(np.tile(ob_v, (128, 1)),
                                        dtype=np.float32),
            "sbt": np.ascontiguousarray(np.tile(s_v, (128, 1)),
                                        dtype=np.float32),
            "labels": np.ascontiguousarray(seizure_labels[b], dtype=np.float32),
            "cosT": cosT, "sinT": sinT,
            "ident": ident, "ident32": ident32, "pswap": psw,
        })
    return in_maps


def run(inputs, trace=False, trace_cores=None):
    x = np.asarray(inputs["x"])
    mask = np.asarray(inputs["mask"])
    # this kernel specializes the additive mask to the causal prefill mask
    causal = np.where(np.tril(np.ones((S, S), dtype=bool)), 0.0, NEG_INF
                      ).astype(np.float32)
    if not np.array_equal(mask, causal):
        raise NotImplementedError("kernel specialized for causal prefill mask")

    in_maps = _prep_inputs(
        x, np.asarray(inputs["freqs_cis"]), np.asarray(inputs["seizure_labels"]),
        np.asarray(inputs["wq"]), np.asarray(inputs["wk"]),
        np.asarray(inputs["wv"]), np.asarray(inputs["wo"]),
        np.asarray(inputs["ln1_w"]), np.asarray(inputs["ln1_b"]),
        np.asarray(inputs["ln2_w"]), np.asarray(inputs["ln2_b"]))

    if "nc" not in _CACHED:
        _CACHED["nc"] = build_nc()
    nc = _CACHED["nc"]

    kw = {}
    if trace:
        kw = dict(trace=True,
                  trace_cores=trace_cores if trace_cores is not None else [0])
    res = run_bass_kernel_spmd(nc, in_maps, core_ids=list(range(8)), **kw)

    shards = [res.results[cid]["out"] for cid in range(8)]
    full = np.empty((B, S, DIM), dtype=np.float32)
    for cid in range(8):
        b, g = divmod(cid, 4)
        full[b, :, DQ_LOC * g:DQ_LOC * (g + 1)] = \
            shards[cid].astype(np.float32)
    return full, res


def kernel(**inputs) -> np.ndarray:
    out, _ = run(inputs, trace=False)
    return out


# revision 33
# speedup vs baseline: 1.0253x; 1.0253x over previous
"""Distributed Trainium2 Bass kernel for a dense-transformer attention block.

Sharding (8 NeuronCores): core cid = 4*b + g
  - b = batch index (B=2), g = kv-head group (N_KV_HEADS=4)
  - Each core: LN1(x[b]) -> its 4 query heads + its 1 kv head (column
    parallel wq/wk/wv), RoPE, causal GQA attention with pre-ictal bias,
    per-q-tile AllGather of attention outputs (groups [0..3], [4..7]),
    LN2 via gathered partial stats, column-parallel wo -> output columns
    [512g:512g+512].  Host concatenates the 8 output shards.

Schedule: everything is emitted interleaved so the PE never idles
(TRN2 PE p-state drops to half clock after any bubble):
  - mask build first (needs only the 8KB labels tensor), overlapping the
    x input DMA;
  - LN1 per 4-tile group as x lands (stats split DVE bn_stats / ACT
    accum-sums), then that group's K/V/Q projections + RoPE;
  - attention per q-tile t right after its kv group is projected
    (2 heads per pass so PSUM fits), diag/prev-diag bias+causal mask
    preloaded into PSUM via ident matmuls;
  - per-tile outputs are transposed pre-gather (4x cheaper than post)
    and AllGathered together with the tile's partial LN2 bn_stats;
  - wo runs on the RAW gathered attention:  out = rs*(a@W) - rs*mu*s + ob
    (s = colsums of W), so the output projection never waits for the
    LN2 normalization; the per-row rs/mu correction is applied after.
  - rsqrt = DVE reciprocal + ACT Sqrt batched per 4 tiles (avoids
    activation-table thrash against the softmax Exp).

Compute dtype: bf16 matmul operands, f32 PSUM accumulation, f32 softmax/LN.
"""

import math
from contextlib import ExitStack

import numpy as np
import ml_dtypes

import concourse.bass as bass
import concourse.bacc as bacc
import concourse.mybir as mybir
import concourse.tile as tile
from concourse.bass_utils import run_bass_kernel_spmd

# Problem constants (hardcoded per spec nn_Attention_36120674959366)
B = 2
S = 2048
DIM = 2048
N_HEADS = 16
N_KV_HEADS = 4
HEAD_DIM = 128
NH_LOC = N_HEADS // N_KV_HEADS  # 4 q-heads per core
DQ_LOC = NH_LOC * HEAD_DIM      # 512
PRE_ICTAL_WINDOW = 10
PRE_ICTAL_BIAS = 2.0
LN_EPS = 1e-5
NEG_INF = -1e9

SQD = math.sqrt(HEAD_DIM)           # 11.3137085
INV_SQD = 1.0 / SQD
BIAS_SCALED = PRE_ICTAL_BIAS * SQD  # 22.627417
NEG_SCALED = NEG_INF * SQD          # -1.13137085e10

NT = S // 128                        # 16 tiles of 128 rows
NC = DIM // 128                      # 16 dim chunks

F32 = mybir.dt.float32
BF16 = mybir.dt.bfloat16

# bounce buffer layout: [attn_cT: 4*128*128 bf16][st6: 128*6 f32 = 1536 bf16]
BNC_DATA = 4 * 128 * 128             # 65536 bf16 elements
BNC_ST6 = 128 * 6 * 2                # f32 viewed as 2x bf16
BNC_TOT = BNC_DATA + BNC_ST6

WO_LAG = 7

_CACHED = {}


def build_nc():
    nc = bacc.Bacc("TRN2", target_bir_lowering=False, debug=False, num_devices=8)

    # ---- kernel I/O (per-core shards; same graph on all 8 cores) ----
    xs = nc.dram_tensor("xs", [S, DIM], BF16, kind="ExternalInput")
    wqT = nc.dram_tensor("wqT", [128, NC * DQ_LOC], BF16, kind="ExternalInput")
    wkT = nc.dram_tensor("wkT", [128, NC * HEAD_DIM], BF16, kind="ExternalInput")
    wvT = nc.dram_tensor("wvT", [128, NC * HEAD_DIM], BF16, kind="ExternalInput")
    woT = nc.dram_tensor("woT", [128, NC * DQ_LOC], BF16, kind="ExternalInput")
    qb = nc.dram_tensor("qb", [128, NH_LOC], F32, kind="ExternalInput")
    kb = nc.dram_tensor("kb", [128, 1], F32, kind="ExternalInput")
    vbt = nc.dram_tensor("vbt", [128, HEAD_DIM], BF16, kind="ExternalInput")
    obt = nc.dram_tensor("obt", [128, DQ_LOC], F32, kind="ExternalInput")
    sbt = nc.dram_tensor("sbt", [128, DQ_LOC], F32, kind="ExternalInput")
    labels = nc.dram_tensor("labels", [S], F32, kind="ExternalInput")
    cosT = nc.dram_tensor("cosT", [HEAD_DIM, S], BF16, kind="ExternalInput")
    sinT = nc.dram_tensor("sinT", [HEAD_DIM, S], BF16, kind="ExternalInput")
    ident = nc.dram_tensor("ident", [128, 128], BF16, kind="ExternalInput")
    ident32 = nc.dram_tensor("ident32", [128, 128], F32, kind="ExternalInput")
    pswap = nc.dram_tensor("pswap", [128, 128], BF16, kind="ExternalInput")
    out = nc.dram_tensor("out", [S, DQ_LOC], BF16, kind="ExternalOutput")

    AF = mybir.ActivationFunctionType
    OP = mybir.AluOpType

    with tile.TileContext(nc) as tc, ExitStack() as st:
        pc = st.enter_context(tc.tile_pool(name="const", bufs=1))
        dr = st.enter_context(tc.tile_pool(name="dr", bufs=1, space="DRAM"))
        ps = st.enter_context(tc.tile_pool(name="ps", bufs=1, space="PSUM"))

        def meg(name):
            return ps.tile([128, 512], F32, tag="meg", bufs=3, name=name)

        def big(name):
            return meg(name)[:, 0:512]

        def pv2(name):
            # two packed [129]-wide softmax-PV accumulators in one bank
            return ps.tile([128, 320], F32, tag="pv2", bufs=3, name=name)

        def tpt(name):
            return ps.tile([128, 512], BF16, tag="tp", bufs=2, name=name)

        # ================= input DMAs =================
        # tiny consts + labels first (mask build starts immediately), then
        # x tiles interleaved with weights so LN1 streams while weights land.
        lab_sb = pc.tile([1, S], F32, tag="lab")
        nc.sync.dma_start(out=lab_sb[:, :],
                          in_=labels.ap().rearrange("(o s) -> o s", o=1))
        ident_sb = pc.tile([128, 128], BF16, tag="ident")
        nc.sync.dma_start(out=ident_sb[:, :], in_=ident[:, :])
        ident32_sb = pc.tile([128, 128], F32, tag="ident32")
        nc.sync.dma_start(out=ident32_sb[:, :], in_=ident32[:, :])
        pswap_sb = pc.tile([128, 128], BF16, tag="pswap")
        nc.sync.dma_start(out=pswap_sb[:, :], in_=pswap[:, :])
        qb_sb = pc.tile([128, NH_LOC], F32, tag="qb")
        nc.sync.dma_start(out=qb_sb[:, :], in_=qb[:, :])
        kb_sb = pc.tile([128, 1], F32, tag="kb")
        nc.sync.dma_start(out=kb_sb[:, :], in_=kb[:, :])
        vb_sb = pc.tile([128, HEAD_DIM], BF16, tag="vb")
        nc.sync.dma_start(out=vb_sb[:, :], in_=vbt[:, :])

        # x tiles stream on the Scalar engine's DMA queue so the weight
        # loads (sync queue) and mask ops (gpsimd/DVE) never stall them.
        pxt = st.enter_context(tc.tile_pool(name="pxt", bufs=1))
        xt_tiles = []
        for i in range(NT):
            xt = pxt.tile([128, DIM], BF16, tag="xt", bufs=6, name=f"xt{i}")
            xt_tiles.append(xt)

        def xt_dma(i):
            nc.scalar.dma_start(out=xt_tiles[i][:, :],
                                in_=xs[128 * i:128 * i + 128, :])

        for i in range(6):
            xt_dma(i)
        cos_sb = pc.tile([128, S], BF16, tag="cos")
        nc.sync.dma_start(out=cos_sb[:, :], in_=cosT[:, :])
        sin_sb = pc.tile([128, S], BF16, tag="sin")
        nc.sync.dma_start(out=sin_sb[:, :], in_=sinT[:, :])

        pw = st.enter_context(tc.tile_pool(name="qkvw", bufs=1))
        wk_sb = pw.tile([128, NC, HEAD_DIM], BF16, tag="wk")
        nc.sync.dma_start(
            out=wk_sb.rearrange("p c n -> p (c n)"), in_=wkT[:, :])
        wv_sb = pw.tile([128, NC, HEAD_DIM], BF16, tag="wv")
        nc.sync.dma_start(
            out=wv_sb.rearrange("p c n -> p (c n)"), in_=wvT[:, :])
        wq_sb = pw.tile([128, NC, DQ_LOC], BF16, tag="wq")
        nc.sync.dma_start(
            out=wq_sb.rearrange("p c n -> p (c n)"), in_=wqT[:, :])
        ob_sb = pc.tile([128, DQ_LOC], F32, tag="ob")
        nc.sync.dma_start(out=ob_sb[:, :], in_=obt[:, :])
        s_sb = pc.tile([128, DQ_LOC], F32, tag="s")
        nc.sync.dma_start(out=s_sb[:, :], in_=sbt[:, :])
        wo_sb = pc.tile([128, NC, DQ_LOC], BF16, tag="wo")
        nc.sync.dma_start(
            out=wo_sb.rearrange("p c n -> p (c n)"), in_=woT[:, :])

        ones_col = pc.tile([1, 128], BF16, tag="ones_col")
        nc.vector.memset(ones_col[:, :], 1.0)

        # ============ mask build ====
        # mgd[kt]: diag-tile bias+causal for q-tile kt; mgp[kt]: bias for
        # q-tile kt+1 vs key-tile kt.  [k partitions, q free], pre-scaled
        # by sqrt(d) (softmax exp applies 1/sqrt(d) to the whole PSUM).
        # The label scan is emitted after group-0 LN1 (keeps the DVE free
        # at t=0); per-tile mask slots are built just-in-time, one tile
        # ahead of the attention that consumes them.
        mgd = pc.tile([128, NT, 128], BF16, tag="mgd")
        mgp = pc.tile([128, NT, 128], BF16, tag="mgp")
        colv = pc.tile([128, NT], F32, tag="colv")
        # cumsum counts are small ints -> exact in bf16
        csrow = pc.tile([1, S + 12], BF16, tag="csrow")

        def emit_scan():
            zrow = pcs.tile([1, S], BF16, tag="zrow")
            nc.vector.memset(zrow[:, :], 0.0)
            nc.vector.memset(csrow[:, 0:1], 0.0)
            nc.vector.tensor_tensor_scan(
                out=csrow[:, 1:S + 1], data0=lab_sb[:, :], data1=zrow[:, :],
                initial=0.0, op0=OP.add, op1=OP.add)
            for j in range(11):
                nc.vector.tensor_copy(csrow[:, S + 1 + j:S + 2 + j],
                                      csrow[:, S:S + 1])
            # spill to DRAM, reload shifted as [NT,128], transpose via PE:
            #   colv[p, kt] = cs[min(k+10, S-1)], k = 128*kt + p
            csbuf = dr.tile([S + 12], BF16)
            nc.sync.dma_start(out=csbuf.rearrange("(o s) -> o s", o=1),
                              in_=csrow[:, :])
            cs16 = pcs.tile([NT, 128], BF16, tag="cs16")
            nc.sync.dma_start(
                out=cs16[:, :],
                in_=csbuf[11:11 + S].rearrange("(t p) -> t p", p=128))
            colv_ps = big("colv_ps")
            nc.tensor.matmul(colv_ps[:, 0:NT], lhsT=cs16[:, :],
                             rhs=ident_sb[0:NT, 0:NT], start=True, stop=True)
            nc.vector.tensor_copy(colv[:, :], colv_ps[:, 0:NT])
            pcs.release()

        def emit_mask(t):
            # rb[p, j] = cs[128*t + j - 1]  (q coordinates of tile t)
            rb = big(f"rb{t}")
            nc.tensor.matmul(rb[:, 0:128], lhsT=ones_col[:, :],
                             rhs=csrow[:, 128 * t:128 * t + 128],
                             start=True, stop=True)
            sl = mgd[:, t, :]
            nc.vector.tensor_scalar(
                out=sl, in0=rb[:, 0:128],
                scalar1=colv[:, t:t + 1], scalar2=BIAS_SCALED,
                op0=OP.is_lt, op1=OP.mult)
            nc.gpsimd.affine_select(
                out=sl, in_=sl,
                compare_op=OP.is_ge, fill=NEG_SCALED,
                base=0, channel_multiplier=-1, pattern=[[1, 128]])
            if t >= 1:
                sl = mgp[:, t - 1, :]
                nc.vector.tensor_scalar(
                    out=sl, in0=rb[:, 0:128],
                    scalar1=colv[:, t - 1:t], scalar2=BIAS_SCALED,
                    op0=OP.is_lt, op1=OP.mult)

        # ================= persistent attention tensors =================
        pqkv = st.enter_context(tc.tile_pool(name="qkv", bufs=1))
        qT = pqkv.tile([128, NH_LOC, S], BF16, tag="qT")
        kT = pqkv.tile([128, S], BF16, tag="kT")
        v_aug = pqkv.tile([128, NT, 132], BF16, tag="v_aug")
        nc.gpsimd.memset(v_aug[:, :, 128:129], 1.0)

        p1 = st.enter_context(tc.tile_pool(name="ln1t", bufs=1))
        ptmp = st.enter_context(tc.tile_pool(name="ln1tmp", bufs=1))
        prope = st.enter_context(tc.tile_pool(name="rope", bufs=1))
        pat = st.enter_context(tc.tile_pool(name="attn", bufs=1))
        pln2 = st.enter_context(tc.tile_pool(name="ln2", bufs=1))
        pcs = tc.alloc_tile_pool(name="csum", bufs=1)

        # ================= LN1 for one row-tile =================
        def emit_ln1(i):
            if i + 6 < NT:
                xt_dma(i + 6)
            xt = xt_tiles[i]
            mu = ptmp.tile([128, 1], F32, tag="mu", bufs=4, name=f"mu{i}")
            var = ptmp.tile([128, 1], F32, tag="var", bufs=4)
            if i % 2 == 0:
                st6 = ptmp.tile([128, 4, 6], F32, tag="st6", bufs=2)
                for a4 in range(4):
                    nc.vector.bn_stats(st6[:, a4, :],
                                       xt[:, 512 * a4:512 * a4 + 512])
                mv = ptmp.tile([128, 2], F32, tag="mv", bufs=2)
                nc.vector.bn_aggr(mv[:, :], st6[:, :, :])
                nc.vector.tensor_copy(mu[:, :], mv[:, 0:1])
                nc.vector.tensor_copy(var[:, :], mv[:, 1:2])
            else:
                scr = ptmp.tile([128, DIM], BF16, tag="scr", bufs=1)
                s1 = ptmp.tile([128, 1], F32, tag="s1", bufs=2)
                s2 = ptmp.tile([128, 1], F32, tag="s2", bufs=2)
                nc.scalar.activation(scr[:, :], xt[:, :], AF.Copy,
                                     accum_out=s1[:, :])
                nc.scalar.activation(scr[:, :], xt[:, :], AF.Square,
                                     accum_out=s2[:, :])
                nc.vector.tensor_scalar_mul(mu[:, :], s1[:, :], 1.0 / DIM)
                musq = ptmp.tile([128, 1], F32, tag="musq", bufs=2)
                nc.vector.tensor_tensor(musq[:, :], mu[:, :], mu[:, :],
                                        op=OP.mult)
                nc.vector.scalar_tensor_tensor(
                    out=var[:, :], in0=s2[:, :], scalar=1.0 / DIM,
                    in1=musq[:, :], op0=OP.mult, op1=OP.subtract)
            rv = ptmp.tile([128, 1], F32, tag="rv", bufs=4, name=f"rv{i}")
            nc.vector.tensor_scalar_add(rv[:, :], var[:, :], LN_EPS)
            nc.vector.reciprocal(rv[:, :], rv[:, :])
            return mu, rv

        # ================= projections + RoPE for one sg strip ==========
        def rope_block(dst, w_sb, h, sg, lnT):
            # dst: [128, 512] slice of qT/kT; raw = W.T@ln1T + bias, then
            # rotary combine.  pswap output consumed from PSUM by DVE.
            lo = 512 * sg
            pq = big(f"pq_{sg}_{h}")
            for c in range(NC):
                lhsT = w_sb[:, c, :] if h is None \
                    else w_sb[:, c, 128 * h:128 * h + 128]
                nc.tensor.matmul(pq, lhsT=lhsT,
                                 rhs=lnT[:, c, :],
                                 start=(c == 0), stop=(c == NC - 1))
            raw = prope.tile([128, 512], BF16, tag="raw", bufs=2)
            bias_ap = kb_sb[:, 0:1] if h is None else qb_sb[:, h:h + 1]
            nc.scalar.activation(raw[:, :], pq, AF.Identity,
                                 bias=bias_ap)
            pw2 = big(f"pw2_{sg}_{h}")
            nc.tensor.matmul(pw2, lhsT=pswap_sb[:, :], rhs=raw[:, :],
                             start=True, stop=True)
            t1 = prope.tile([128, 512], BF16, tag="t1", bufs=2)
            nc.vector.tensor_mul(t1[:, :], raw[:, :], cos_sb[:, lo:lo + 512])
            t2 = prope.tile([128, 512], BF16, tag="t2", bufs=2)
            nc.vector.tensor_tensor(t2[:, :], pw2,
                                    sin_sb[:, lo:lo + 512], op=OP.mult)
            nc.vector.tensor_add(dst, t1[:, :], t2[:, :])

        def emit_half(g4, half, lnT):
            # LN1 + transposes for tiles 4*g4+2*half .. +1 (DVE tile then
            # ACT tile run on separate engines in parallel)
            murs = [emit_ln1(4 * g4 + 2 * half + j) for j in range(2)]
            rss = []
            for j in range(2):
                rs = ptmp.tile([128, 1], F32, tag="rs", bufs=4,
                               name=f"rs{4 * g4 + 2 * half + j}")
                nc.scalar.activation(rs[:, :], murs[j][1][:, :], AF.Sqrt)
                rss.append(rs)
            xh_tiles = []
            for j in range(2):
                i = 4 * g4 + 2 * half + j
                xh = ptmp.tile([128, DIM], BF16, tag="xh", bufs=4)
                nc.vector.tensor_scalar(
                    out=xh[:, :], in0=xt_tiles[i][:, :],
                    scalar1=murs[j][0][:, :], scalar2=rss[j][:, :],
                    op0=OP.subtract, op1=OP.mult)
                xh_tiles.append(xh)
            for c in range(NC):
                pt = tpt(f"lt_{g4}_{half}_{c}")
                for j in range(2):
                    nc.tensor.transpose(
                        pt[:, 128 * j:128 * j + 128],
                        xh_tiles[j][:, 128 * c:128 * c + 128],
                        ident_sb[:, :])
                nc.scalar.activation(lnT[:, c, 256 * half:256 * half + 256],
                                     pt[:, 0:256], AF.Copy)

        def emit_group_proj(g4):
            lnT = p1.tile([128, NC, 512], BF16, tag="ln1T", bufs=1,
                          name=f"ln1T{g4}")
            emit_half(g4, 0, lnT)
            emit_half(g4, 1, lnT)
            # K strip
            rope_block(kT[:, 512 * g4:512 * g4 + 512], wk_sb, None, g4, lnT)
            # V tiles
            for j4 in range(4):
                i = 4 * g4 + j4
                pv = big(f"pv_{i}")
                for c in range(NC):
                    nc.tensor.matmul(
                        pv[:, 0:128],
                        lhsT=lnT[:, c, 128 * j4:128 * j4 + 128],
                        rhs=wv_sb[:, c, :],
                        start=(c == 0), stop=(c == NC - 1))
                nc.vector.tensor_add(v_aug[:, i, 0:128], pv[:, 0:128],
                                     vb_sb[:, :])
            # Q strips
            for h in range(NH_LOC):
                rope_block(qT[:, h, 512 * g4:512 * g4 + 512], wq_sb, h,
                           g4, lnT)

        # ================= attention for one q-tile =================
        bounce_ins = [None] * (NT // 2)
        bounce_outs = [None] * (NT // 2)
        attn_cs = {}

        def emit_attn(t):
            attn_c = pat.tile([128, 4, 128], BF16, tag="attn_c", bufs=3,
                              name=f"attn_c{t}")
            pva = pv2(f"pva_{t}")
            pvb = pv2(f"pvb_{t}")
            # four packed softmax-PV accumulators: (bank, region) per head
            pvp = [pva[:, 0:129], pva[:, 160:289],
                   pvb[:, 0:129], pvb[:, 160:289]]
            def emit_qk(k):
                mega = meg(f"mega_{t}_{k}")
                if k >= t - 1:
                    mt = mgd if k == t else mgp
                    for hp in range(4):
                        nc.tensor.matmul(
                            mega[:, 128 * hp:128 * hp + 128],
                            lhsT=ident_sb[:, :],
                            rhs=mt[:, k, :],
                            start=(hp == 0), stop=False,
                            skip_group_check=True)
                    nc.tensor.matmul(
                        mega, lhsT=kT[:, 128 * k:128 * k + 128],
                        rhs=qT[:, :, 128 * t:128 * t + 128],
                        start=False, stop=True,
                        skip_group_check=True)
                else:
                    nc.tensor.matmul(
                        mega, lhsT=kT[:, 128 * k:128 * k + 128],
                        rhs=qT[:, :, 128 * t:128 * t + 128],
                        start=True, stop=True)
                return mega

            def emit_exp(k, mega):
                pt = pat.tile([128, 512], BF16, tag="pt_sm", bufs=3)
                nc.scalar.activation(pt[:, :], mega, AF.Exp, scale=INV_SQD)
                return pt

            def emit_pv(k, pt):
                for hp in range(4):
                    # one accumulation group per PSUM bank: start only
                    # on the bank's very first write (h even, k==0)
                    nc.tensor.matmul(
                        pvp[hp],
                        lhsT=pt[:, 128 * hp:128 * hp + 128],
                        rhs=v_aug[:, k, 0:129],
                        start=(k == 0 and hp % 2 == 0),
                        stop=(k == t and hp % 2 == 1),
                        skip_group_check=True)

            # software-pipelined: QK(k+1) issues on the PE before PV(k)
            # so the PE never waits for exp(k) on the ACT engine.
            prev = None
            for k in range(t + 1):
                mega = emit_qk(k)
                pt = emit_exp(k, mega)
                if prev is not None:
                    emit_pv(*prev)
                prev = (k, pt)
            emit_pv(*prev)
            for hp in range(4):
                rcp = pat.tile([128, 1], F32, tag="rcp", bufs=4)
                nc.vector.reciprocal(rcp[:, :], pvp[hp][:, 128:129])
                nc.vector.tensor_single_scalar(
                    out=attn_c[:, hp, :],
                    in_=pvp[hp][:, 0:128],
                    scalar=rcp[:, :], op=OP.mult)
            attn_cs[t] = attn_c

        def emit_attn_flush(t):
            # deferred one tile so the PE transposes never head-of-line
            # block behind the tile's DVE normalize chain
            attn_c = attn_cs.pop(t)
            # partial LN2 stats for this core's 512 columns
            st6a = pat.tile([128, 6], F32, tag="st6a", bufs=2)
            nc.vector.bn_stats(st6a[:, :],
                               attn_c.rearrange("p h v -> p (h v)"))
            st6b = pat.tile([128, 6, 2], BF16, tag="st6b", bufs=2)
            nc.vector.tensor_copy(
                st6b.rearrange("p a b -> p (a b)").bitcast(F32),
                st6a[:, :])
            # pre-gather transpose: [q, (h,v)] -> [(h,v), q] in 128-chunks
            tp = tpt(f"at_{t}")
            for c4 in range(4):
                nc.tensor.transpose(tp[:, 128 * c4:128 * c4 + 128],
                                    attn_c[:, c4, :], ident_sb[:, :])
            attn_cT = pat.tile([128, 4, 128], BF16, tag="attn_cT", bufs=2)
            nc.vector.tensor_copy(
                attn_cT.rearrange("p c q -> p (c q)"), tp[:, :])
            if t % 2 == 0:
                bounce_ins[t // 2] = dr.tile([2 * BNC_TOT], BF16,
                                             name=f"bin{t // 2}")
            bounce_in = bounce_ins[t // 2]
            off = (t % 2) * BNC_TOT
            nc.sync.dma_start(
                out=bounce_in[off:off + BNC_DATA].rearrange(
                    "(c p q) -> p c q", c=4, p=128),
                in_=attn_cT[:, :, :])
            nc.gpsimd.dma_start(
                out=bounce_in[off + BNC_DATA:off + BNC_TOT].rearrange(
                    "(p n) -> p n", p=128),
                in_=st6b.rearrange("p a b -> p (a b)"))

        def emit_ag(pr):
            bounce_out = dr.tile([4, 2 * BNC_TOT], BF16, name=f"bout{pr}")
            nc.gpsimd.collective_compute(
                "AllGather",
                mybir.AluOpType.bypass,
                replica_groups=[[0, 1, 2, 3], [4, 5, 6, 7]],
                ins=[bounce_ins[pr][:].opt()],
                outs=[bounce_out[:, :].opt()],
            )
            bounce_outs[pr] = bounce_out


        # ============ gathered-side LN2 stats + raw wo ============
        rvs = {}
        posbs = {}
        prefetched = {}

        def emit_prefetch(t):
            bo = bounce_outs[t // 2]
            off = (t % 2) * BNC_TOT
            afsT = pln2.tile([128, 16, 128], BF16, tag="afsT", bufs=3,
                             name=f"afsT{t}")
            for g in range(4):
                nc.sync.dma_start(
                    out=afsT[:, 4 * g:4 * g + 4, :],
                    in_=bo[g, off:off + BNC_DATA].rearrange(
                        "(c p q) -> p c q", c=4, p=128))
            st6g = pln2.tile([128, 4, 6, 2], BF16, tag="st6g", bufs=3)
            nc.gpsimd.dma_start(
                out=st6g.rearrange("p g a b -> p g (a b)"),
                in_=bo[:, off + BNC_DATA:off + BNC_TOT].rearrange(
                    "g (p n) -> p g n", p=128))
            prefetched[t] = (afsT, st6g)

        def emit_ln2mm(t):
            afsT, st6g = prefetched.pop(t)
            st6f = pln2.tile([128, 4, 6], F32, tag="st6f", bufs=2)
            nc.vector.tensor_copy(
                st6f.rearrange("p g a -> p (g a)"),
                st6g.rearrange("p g a b -> p (g a b)").bitcast(F32))
            mv2 = pln2.tile([128, 2], F32, tag="mv2", bufs=2)
            nc.vector.bn_aggr(mv2[:, :], st6f[:, :, :])
            rv2 = pln2.tile([128, 1], F32, tag="rv2", bufs=9, name=f"rv2_{t}")
            nc.vector.tensor_scalar_add(rv2[:, :], mv2[:, 1:2], LN_EPS)
            nc.vector.reciprocal(rv2[:, :], rv2[:, :])
            rvs[t] = rv2
            nmu = pln2.tile([128, 1], F32, tag="nmu", bufs=2)
            nc.vector.tensor_scalar_mul(nmu[:, :], mv2[:, 0:1], -1.0)
            po = big(f"po_{t}")
            for c in range(NC):
                nc.tensor.matmul(po, lhsT=afsT[:, c, :],
                                 rhs=wo_sb[:, c, :],
                                 start=(c == 0), stop=(c == NC - 1))
            posb = pln2.tile([128, DQ_LOC], BF16, tag="posb", bufs=10,
                             name=f"posb{t}")
            nc.vector.scalar_tensor_tensor(
                out=posb[:, :], in0=s_sb[:, :], scalar=nmu[:, :],
                in1=po, op0=OP.mult, op1=OP.add)
            posbs[t] = posb

        def emit_finish(ts):
            # batched ACT Sqrt (one table switch per batch), then the
            # rs correction + output DMA for each tile in the batch.
            rss = {}
            for t in ts:
                rs = pln2.tile([128, 1], F32, tag="rs2", bufs=9,
                               name=f"rs2_{t}")
                nc.scalar.activation(rs[:, :], rvs[t][:, :], AF.Sqrt)
                rss[t] = rs
            for t in ts:
                osb = pln2.tile([128, DQ_LOC], BF16, tag="osb", bufs=2,
                                name=f"osb{t}")
                nc.vector.scalar_tensor_tensor(
                    out=osb[:, :], in0=posbs[t][:, :], scalar=rss[t][:, :],
                    in1=ob_sb[:, :], op0=OP.mult, op1=OP.add)
                nc.gpsimd.dma_start(out=out[128 * t:128 * t + 128, :],
                                    in_=osb[:, :])

        # ================= main interleaved schedule =================
        # tiny dummy collective up front: absorbs the one-time CC ring
        # warmup (~20us) while the PE is still in LN1/proj.
        warm_in = dr.tile([2 * BNC_TOT], BF16, name="warm_in")
        warm_sb = pat.tile([128, 1048], BF16, tag="warm")
        nc.vector.memset(warm_sb[:, :], 0.0)
        nc.gpsimd.dma_start(
            out=warm_in.rearrange("(p n) -> p n", p=128),
            in_=warm_sb[:, :])
        warm_out = dr.tile([4, 2 * BNC_TOT], BF16, name="warm_out")
        nc.gpsimd.collective_compute(
            "AllGather", mybir.AluOpType.bypass,
            replica_groups=[[0, 1, 2, 3], [4, 5, 6, 7]],
            ins=[warm_in[:].opt()], outs=[warm_out[:, :].opt()])

        for g4 in range(4):
            emit_group_proj(g4)
            if g4 == 0:
                emit_scan()
                emit_mask(0)
                emit_mask(1)
            for j4 in range(4):
                t = 4 * g4 + j4
                if t + 2 < NT:
                    emit_mask(t + 2)
                emit_attn(t)
                if t >= 1:
                    emit_attn_flush(t - 1)
                    if t % 2 == 0:
                        emit_ag((t - 1) // 2)
                if t >= WO_LAG - 2:
                    emit_prefetch(t - WO_LAG + 2)
                if t >= WO_LAG:
                    emit_ln2mm(t - WO_LAG)
                if t - WO_LAG in (3, 7):
                    emit_finish(range(t - WO_LAG - 3, t - WO_LAG + 1))
        emit_attn_flush(15)
        emit_ag(7)
        emit_prefetch(11)
        emit_ln2mm(9)
        emit_prefetch(12)
        emit_ln2mm(10)
        emit_prefetch(13)
        emit_ln2mm(11)
        emit_prefetch(14)
        emit_ln2mm(12)
        emit_finish(range(8, 12))
        emit_prefetch(15)
        emit_ln2mm(13)
        emit_ln2mm(14)
        emit_ln2mm(15)
        emit_finish(range(12, 16))

    nc.compile()
    return nc


def _prep_inputs(x, freqs_cis, seizure_labels, wq, wk, wv, wo,
                 ln1_w, ln1_b, ln2_w, ln2_b):
    bf16 = ml_dtypes.bfloat16
    cos = np.asarray(freqs_cis[..., 0], dtype=np.float32)  # [S, 64]
    sin = np.asarray(freqs_cis[..., 1], dtype=np.float32)
    cosT = np.ascontiguousarray(np.repeat(cos.T, 2, axis=0), dtype=bf16)
    sgn = np.where(np.arange(HEAD_DIM) % 2 == 0, -1.0, 1.0).astype(np.float32)
    sinT = np.ascontiguousarray(np.repeat(sin.T, 2, axis=0) * sgn[:, None],
                                dtype=bf16)
    ident = np.eye(128, dtype=bf16)
    ident32 = np.eye(128, dtype=np.float32)
    psw = np.zeros((128, 128), dtype=np.float32)
    idx = np.arange(128)
    psw[idx ^ 1, idx] = 1.0  # out[m, s] = sum_k psw[k, m] * in[k, s] = in[m^1, s]
    psw = psw.astype(bf16)

    # fold LN affine weights into the projection weights (host-side
    # preprocessing, standard inference-time weight folding):
    #   ln(x)@W.T = xhat@(W*w).T + b@W.T
    w1 = np.asarray(ln1_w, np.float64)
    b1 = np.asarray(ln1_b, np.float64)
    w2 = np.asarray(ln2_w, np.float64)
    b2 = np.asarray(ln2_b, np.float64)
    in_maps = []
    for cid in range(8):
        b, g = divmod(cid, 4)
        wq_s = np.asarray(wq[DQ_LOC * g:DQ_LOC * (g + 1), :], np.float64)
        wk_s = np.asarray(wk[HEAD_DIM * g:HEAD_DIM * (g + 1), :], np.float64)
        wv_s = np.asarray(wv[HEAD_DIM * g:HEAD_DIM * (g + 1), :], np.float64)
        wo_s = np.asarray(wo[DQ_LOC * g:DQ_LOC * (g + 1), :], np.float64)
        qb_v = (b1 @ wq_s.T).astype(np.float32)         # [512]
        kb_v = (b1 @ wk_s.T).astype(np.float32)         # [128]
        vb_v = (b1 @ wv_s.T).astype(np.float32)         # [128]
        ob_v = (b2 @ wo_s.T).astype(np.float32)         # [512]
        wo_eff = wo_s * w2                              # [512, 2048]
        s_v = wo_eff.sum(axis=1).astype(np.float32)     # [512] colsums of W.T
        def arr(wT):  # [DIM, n] -> [128, NC*n] chunk-major per partition
            n = wT.shape[1]
            return np.ascontiguousarray(
                wT.reshape(NC, 128, n).transpose(1, 0, 2).reshape(128, NC * n),
                dtype=bf16)
        in_maps.append({
            "xs": np.ascontiguousarray(x[b], dtype=bf16),
            "wqT": arr((wq_s * w1).T),
            "wkT": arr((wk_s * w1).T),
            "wvT": arr((wv_s * w1).T),
            "woT": arr(wo_eff.T),
            "qb": np.ascontiguousarray(
                qb_v.reshape(NH_LOC, 128).T, dtype=np.float32),
            "kb": np.ascontiguousarray(kb_v.reshape(128, 1), dtype=np.float32),
            "vbt": np.ascontiguousarray(np.tile(vb_v, (128, 1)), dtype=bf16),
            "obt": np.ascontiguousarray(np.tile(ob_v, (128, 1)),
                                        dtype=np.float32),
            "sbt": np.ascontiguousarray(np.tile(s_v, (128, 1)),
                                        dtype=np.float32),
            "labels": np.ascontiguousarray(seizure_labels[b], dtype=np.float32),
            "cosT": cosT, "sinT": sinT,
            "ident": ident, "ident32": ident32, "pswap": psw,
        })
    return in_maps


def run(inputs, trace=False, trace_cores=None):
    x = np.asarray(inputs["x"])
    mask = np.asarray(inputs["mask"])
    # this kernel specializes the additive mask to the causal prefill mask
    causal = np.where(np.tril(np.ones((S, S), dtype=bool)), 0.0, NEG_INF
                      ).astype(np.float32)
    if not np.array_equal(mask, causal):
        raise NotImplementedError("kernel specialized for causal prefill mask")

    in_maps = _prep_inputs(
        x, np.asarray(inputs["freqs_cis"]), np.asarray(inputs["seizure_labels"]),
        np.asarray(inputs["wq"]), np.asarray(inputs["wk"]),
        np.asarray(inputs["wv"]), np.asarray(inputs["wo"]),
        np.asarray(inputs["ln1_w"]), np.asarray(inputs["ln1_b"]),
        np.asarray(inputs["ln2_w"]), np.asarray(inputs["ln2_b"]))

    if "nc" not in _CACHED:
        _CACHED["nc"] = build_nc()
    nc = _CACHED["nc"]

    kw = {}
    if trace:
        kw = dict(trace=True,
                  trace_cores=trace_cores if trace_cores is not None else [0])
    res = run_bass_kernel_spmd(nc, in_maps, core_ids=list(range(8)), **kw)

    shards = [res.results[cid]["out"] for cid in range(8)]
    full = np.empty((B, S, DIM), dtype=np.float32)
    for cid in range(8):
        b, g = divmod(cid, 4)
        full[b, :, DQ_LOC * g:DQ_LOC * (g + 1)] = \
            shards[cid].astype(np.float32)
    return full, res


def kernel(**inputs) -> np.ndarray:
    out, _ = run(inputs, trace=False)
    return out
